# revision 24
# baseline (speedup 1.0000x reference)
"""Trainium2 Bass kernel for nn_Caps2dMatwo (capsule conv + dual routing).

Strategy (8 NeuronCores, no cross-core communication):
  - Shard: core k handles batch n=k//4, H-rows [32*(k%4), 32*(k%4)+32).
  - PE stage: 3x3 conv + capsule pose/appearance transforms fused into 9
    per-tap matmuls over a permuted 128-channel axis, block-diagonal per
    32-row i-block.  PSUM column layout per i-block: (pa, t, c', k) with
    c'<4 the per-input-capsule u_hat and c'=4 the 0.5*sum_c partial (the
    iteration-1 routing p), then 40 cols of raw j=3 conv taps (t, c'')
    for the coordinate-addition fixup.
  - Routing (3 iterations) runs with R=8 rows batched per instruction:
    pixels on partitions, (row, pa, t, c, ik) on the free axis.  DVE does
    the heavy elementwise work, GPSIMD evacuates PSUM (folding the app
    bias into the evac add), ACT does only Sqrt/Sigmoid.
  - Agreements use raw p with factor scaling deferred past the reduce,
    so the ACT sqrt hides behind the big DVE ops.
  - Output is written w-major per core; the host transposes to
    channel-major when gathering.
"""
import os
from contextlib import ExitStack

import numpy as np

# problem constants (hardcoded per spec)
N, T0, T1 = 2, 4, 8
H, W = 128, 128
PD, AD = 4, 4
Z = 32
NBLK = 360          # psum cols per i-block: 320 (pa,t,c',k) + 40 (t,c'')
ROWS = 32           # output rows per core
P = 128
R = 8               # rows batched per routing group
NG = ROWS // R

_CACHE = {}


# ----------------------------------------------------------------- host prep
def _build_weights(W_conv, W_pos, W_app, b_app):
    """W_eff for the fused conv+pose matmul.

    Column layout within each i-block (360 cols):
      pa*160 + t*20 + c*4 + k   (c<4)  u_hat contribution
      pa*160 + t*20 + 16  + k          0.5 * sum_c u_hat  (iter-1 p)
      320 + t*5 + c             (c<4)  raw j=3 pos conv tap (coord fixup)
      320 + t*5 + 4                    0.5 * sum_c raw tap (p fixup)

    Returns:
      w_in [128, 9, 360]  per-partition weights, partition = i*32+c*8+q*4+j
      bapp [8, 5, 4]      app bias (t, c', k): c'<4 = b_app*colsum(Mapp),
                          c'=4 = 0.5*sum_c of that
    """
    Kc = np.asarray(W_conv, np.float64)[:, :, :, 0, :]          # [c,dy,dx,t1]
    Mpos = np.asarray(W_pos, np.float64).reshape(T0, T1, PD, PD).copy()
    Mpos = Mpos / np.sqrt(np.maximum((Mpos ** 2).sum(axis=2, keepdims=True), 1e-12))
    Mapp = np.asarray(W_app, np.float64).reshape(T0, T1, AD, AD)
    Sapp = Mapp.sum(axis=2)                                      # [c,t,k]

    W_eff = np.zeros((9, 128, 4, NBLK), np.float64)
    for tap in range(9):
        dy, dx = tap // 3, tap % 3
        for i in range(4):
            for c in range(4):
                for q in range(2):
                    for j in range(4):
                        row = i * 32 + c * 8 + q * 4 + j
                        for t in range(q, 8, 2):
                            kpos = Kc[c, dy, dx, t // 2]
                            kapp = Kc[c, dy, dx, 4 + t // 2]
                            for k in range(4):
                                W_eff[tap, row, i, t * 20 + c * 4 + k] = \
                                    kpos * Mpos[c, t, j, k]
                                W_eff[tap, row, i, 160 + t * 20 + c * 4 + k] = \
                                    kapp * Mapp[c, t, j, k]
                                W_eff[tap, row, i, t * 20 + 16 + k] += \
                                    0.5 * kpos * Mpos[c, t, j, k]
                                W_eff[tap, row, i, 160 + t * 20 + 16 + k] += \
                                    0.5 * kapp * Mapp[c, t, j, k]
                            if j == 3:
                                W_eff[tap, row, i, 320 + t * 5 + c] = kpos
                                W_eff[tap, row, i, 320 + t * 5 + 4] += 0.5 * kpos
    # [9, 128, 4, 360] -> [128, 9, 360] picking each partition's own block
    w_in = np.zeros((128, 9, NBLK), np.float32)
    for i in range(4):
        w_in[i * 32:(i + 1) * 32] = W_eff[:, i * 32:(i + 1) * 32, i, :].transpose(1, 0, 2)
    # bias "ones tap" weights: app-half columns get bias/32 (32 partitions
    # of ones contract to exactly bias; /32 is exact in fp16)
    buh = np.einsum('ct,ctk->tck', np.asarray(b_app, np.float64), Sapp)  # [t,c,k]
    wb = np.zeros((NBLK,), np.float64)
    for t in range(8):
        for c in range(4):
            wb[160 + t * 20 + c * 4:160 + t * 20 + c * 4 + 4] = buh[t, c]
        wb[160 + t * 20 + 16:160 + t * 20 + 20] = 0.5 * buh[t].sum(axis=0)
    wb_in = np.broadcast_to((wb / 32.0)[None, :], (128, NBLK))
    return w_in, np.ascontiguousarray(wb_in).astype(np.float32)


def _shard_x(x):
    """x [N,T0,Z,H,W] -> list of 8 arrays [128, 34*130] (permuted channels)."""
    xp = np.zeros((N, T0, Z, H + 2, W + 2), np.float32)
    xp[:, :, :, 1:H + 1, 1:W + 1] = np.asarray(x, np.float32)
    # z = q*16 + i*4 + j ; partition = i*32 + c*8 + q*4 + j
    xq = xp.reshape(N, T0, 2, 4, 4, H + 2, W + 2)                # n c q i j h w
    xperm = np.ascontiguousarray(xq.transpose(0, 3, 1, 2, 4, 5, 6)
                                 ).reshape(N, 128, H + 2, W + 2)
    shards = []
    for core in range(8):
        n, rb = core // 4, (core % 4) * 32
        shards.append(np.ascontiguousarray(
            xperm[n, :, rb:rb + 34, :]).reshape(128, 34 * 130))
    return shards


# ------------------------------------------------------------- bass module
def _build_module():
    import concourse.bass as bass
    import concourse.tile as tile
    import concourse.mybir as mybir
    from concourse import bacc

    f32 = mybir.dt.float32
    f16 = mybir.dt.float16
    AX = mybir.AxisListType.X
    OP = mybir.AluOpType
    AF = mybir.ActivationFunctionType

    nc = bacc.Bacc("TRN2", num_devices=8, debug=False)
    x_d = nc.dram_tensor("x_shard", [128, 34 * 130], f16, kind="ExternalInput").ap()
    w_d = nc.dram_tensor("w_eff", [128, 9, NBLK], f16, kind="ExternalInput").ap()
    bapp_d = nc.dram_tensor("bapp", [128, NBLK], f16, kind="ExternalInput").ap()
    cxy_d = nc.dram_tensor("cxy", [128, 1 + ROWS], f32, kind="ExternalInput").ap()
    out_d = nc.dram_tensor("out_shard", [128, ROWS * 256], f16,
                           kind="ExternalOutput").ap()

    with tile.TileContext(nc) as tc, ExitStack() as ctx:
        const = ctx.enter_context(tc.tile_pool(name="const", bufs=1))
        grp = ctx.enter_context(tc.tile_pool(name="grp", bufs=2))
        sm = ctx.enter_context(tc.tile_pool(name="sm", bufs=2))
        big = ctx.enter_context(tc.tile_pool(name="big", bufs=1))
        psum = ctx.enter_context(tc.tile_pool(name="psum", bufs=2, space="PSUM"))

        x_sb = const.tile([P, 34, 130], f16)
        nc.sync.dma_start(out=x_sb[:].rearrange("p a b -> p (a b)"), in_=x_d)
        w_sb = const.tile([P, 9, NBLK], f16)
        nc.sync.dma_start(out=w_sb, in_=w_d)
        wb_sb = const.tile([P, NBLK], f16)        # bias/32 "ones tap" weights
        nc.sync.dma_start(out=wb_sb, in_=bapp_d)
        cxy = const.tile([P, 1 + ROWS], f32)      # [cx | cy per row]
        nc.sync.dma_start(out=cxy, in_=cxy_d)
        eps_t = const.tile([P, 1], f32)
        nc.vector.memset(eps_t, 1e-9)
        ones_sb = const.tile([P, 128], f16)
        nc.vector.memset(ones_sb, 1.0)

        st = {}  # per-group live tiles

        def mm_evac(g):
            """PE matmuls + ACT PSUM evacuation for group g's rows."""
            up1 = grp.tile([P, R, 2, 8, 5, 16], f16, tag="up1")
            crw = grp.tile([P, R, 8, 5, 4], f16, tag="crw")
            st[g] = {"up1": up1, "crw": crw}
            for j in range(R):
                r = g * R + j
                ps = psum.tile([P, 4, 512], f32, tag="ps")
                for tap in range(9):
                    dy, dx = tap // 3, tap % 3
                    patch = x_sb[:, r + dy, dx:dx + 128]
                    for i in range(4):
                        nc.tensor.matmul(
                            ps[:, i, 0:NBLK],
                            lhsT=patch[32 * i:32 * (i + 1), :],
                            rhs=w_sb[32 * i:32 * (i + 1), tap, :],
                            start=(tap == 0), stop=False,
                            tile_position=(32 * i, 0))
                for i in range(4):      # bias tap: sum_p 1*(bias/32) = bias
                    nc.tensor.matmul(
                        ps[:, i, 0:NBLK],
                        lhsT=ones_sb[32 * i:32 * (i + 1), :],
                        rhs=wb_sb[32 * i:32 * (i + 1), :],
                        start=False, stop=True,
                        tile_position=(32 * i, 0))
                # evac: (i, (t c'), k) views — 3 free dims on both sides
                src = ps[:, :, 0:320].rearrange("p i (pa tc k) -> p pa i tc k",
                                                pa=2, tc=40)
                dst = up1[:, j].rearrange("p pa t c (i k) -> p pa i (t c) k", i=4)
                nc.scalar.copy(dst[:, 0], src[:, 0])
                nc.scalar.copy(dst[:, 1], src[:, 1])
                nc.scalar.copy(
                    crw[:, j],
                    ps[:, :, 320:360].rearrange("p i (t c) -> p t c i", t=8))

        def prefix(g):
            """GPSIMD: coordinate-addition fixup into u_hat pos (k=0,1).

            The c'=4 (p1) slot rides along: tmp's c''=4 holds the scaled
            0.5*sum_c raw tap, exactly the p1 coordinate term.  All TT ops
            (TensorScalarPtr is ~100x slower on GPSIMD).
            """
            up1, crw = st[g]["up1"], st[g]["crw"]
            tmp = sm.tile([P, 2, R, 8, 5, 4], f16, tag="tmp")
            crf = crw[:].rearrange("p r t c i -> p (r t c i)")
            nc.gpsimd.tensor_mul(
                tmp[:, 0].rearrange("p r t c i -> p (r t c i)"),
                crf, cxy[:, 0:1].broadcast_to((P, R * 160)))
            cyg = (cxy[:, 1 + g * R:1 + (g + 1) * R]
                   .rearrange("p r -> p r ()")
                   .broadcast_to((P, R, 160)))
            nc.gpsimd.tensor_mul(
                tmp[:, 1].rearrange("p r t c i -> p r (t c i)"),
                crw[:].rearrange("p r t c i -> p r (t c i)"), cyg)
            uh_ik = up1[:, :, 0].rearrange("p r t c (i k) -> p r (t c) i k", i=4)
            for k in (0, 1):
                nc.gpsimd.tensor_add(
                    uh_ik[:, :, :, :, k], uh_ik[:, :, :, :, k],
                    tmp[:, k].rearrange("p r t c i -> p r (t c) i"))

        def squash_a(g, ppos, papp, it):
            """Reduces + ACT sqrt (issued early; f finished in squash_b).

            The square runs on GPSIMD for iters 1-2 (latency hidden under
            the following wp/tree ops) to offload the DVE.
            """
            md = sm.tile([P, R, 2, 8], f32, tag=f"md{it}")
            nc.vector.tensor_reduce(out=md[:, :, 0], in_=ppos, axis=AX,
                                    op=OP.max, apply_absolute_value=True)
            sq = big.tile([P, R, 8, 16], f32, tag="sq")
            eng = nc.gpsimd if it < 3 else nc.vector
            eng.tensor_mul(sq, papp, papp)
            s = sm.tile([P, R, 8], f32, tag=f"s{it}")
            nc.vector.tensor_reduce(out=s, in_=sq, axis=AX, op=OP.add)
            sq1 = sm.tile([P, R, 8], f32, tag=f"sq1{it}")
            nc.scalar.activation(sq1, s, AF.Sqrt, bias=eps_t[:, 0:1])
            return md, s, sq1

        def squash_b(g, md, s, sq1):
            """DVE: den = (1+s)*sqrt, f = [1/mx | s/den]."""
            nc.vector.scalar_tensor_tensor(out=md[:, :, 1], in0=s, scalar=1.0,
                                           in1=sq1, op0=OP.add, op1=OP.mult)
            f = sm.tile([P, R, 2, 8], f16, tag="f")
            with nc.allow_low_precision("f consumed in fp16 muls"):
                nc.vector.reciprocal(f[:], md[:])
            nc.vector.tensor_mul(f[:, :, 1], s, f[:, :, 1])
            return f

        def wp_ab(g, pv):
            """DVE: wp = uh*p (bcast over c), ab = sum_ik wp (tree adds)."""
            up1 = st[g]["up1"]
            uh = (up1[:, :, :, :, 0:4, :]
                  .rearrange("p r pa t c ik -> p (r pa t) c ik"))
            pm = (pv.rearrange("p r pa t ik -> p (r pa t) () ik")
                  .broadcast_to((P, R * 16, 4, 16)))
            wp = big.tile([P, R * 16, 4, 16], f16, tag="wp")
            nc.vector.tensor_mul(wp, uh, pm)
            wa = big.tile([P, R * 16, 4, 8], f16, tag="wa")
            nc.vector.tensor_add(wa, wp[:, :, :, 0:8], wp[:, :, :, 8:16])
            wb = big.tile([P, R * 16, 4, 4], f16, tag="wb")
            nc.vector.tensor_add(wb, wa[:, :, :, 0:4], wa[:, :, :, 4:8])
            wc = big.tile([P, R * 16, 4, 2], f16, tag="wc")
            nc.vector.tensor_add(wc, wb[:, :, :, 0:2], wb[:, :, :, 2:4])
            ab = sm.tile([P, R * 16, 4], f32, tag="ab")
            nc.vector.tensor_add(ab, wc[:, :, :, 0], wc[:, :, :, 1])
            return ab

        def badd(g, ab, f, first):
            """DVE: rt = (f_pos*ab_pos)*(f_app*ab_app); b (+)= rt."""
            t1 = sm.tile([P, R, 2, 8, 4], f16, tag="t1")
            nc.vector.tensor_mul(
                t1[:].rearrange("p r pa t c -> p (r pa) t c"),
                ab[:].rearrange("p (rpa t) c -> p rpa t c", t=8),
                f[:].rearrange("p r pa t -> p (r pa) t ()")
                .broadcast_to((P, R * 2, 8, 4)))
            if first:
                b = sm.tile([P, R, 8, 4], f32, tag="b")
                st[g]["b"] = b
                nc.vector.tensor_mul(b, t1[:, :, 0], t1[:, :, 1])
            else:
                b = st[g]["b"]
                rt = sm.tile([P, R, 8, 4], f32, tag="rt")
                nc.vector.tensor_mul(rt, t1[:, :, 0], t1[:, :, 1])
                nc.vector.tensor_add(b, b, rt)

        def sig_p(g, last):
            """ACT sigmoid (ik-expanded) + DVE: p = sum_c uh*r.

            Returns p as [P,R,2,8,16] (pa-major), or for `last` as
            [P,R,8,2,16] (t-major, the output channel layout).
            """
            up1, b = st[g]["up1"], st[g]["b"]
            r2 = sm.tile([P, R, 8, 4], f16, tag="r2")
            nc.scalar.activation(r2, b, AF.Sigmoid)
            m = big.tile([P, R, 2, 8, 4, 16], f16, tag="m")
            for pa in (0, 1):
                for c in range(4):
                    nc.vector.tensor_mul(
                        m[:, :, pa, :, c, :], up1[:, :, pa, :, c, :],
                        r2[:, :, :, c].unsqueeze(3)
                        .broadcast_to((P, R, 8, 16)))
            mm = m[:].rearrange("p r pa t c ik -> p (r pa) t c ik")
            ta = big.tile([P, R, 2, 8, 16], f16, tag="ta")
            tb = big.tile([P, R, 2, 8, 16], f16, tag="tb")
            tam = ta[:].rearrange("p r pa t ik -> p (r pa) t ik")
            tbm = tb[:].rearrange("p r pa t ik -> p (r pa) t ik")
            nc.vector.tensor_add(tam, mm[:, :, :, 0], mm[:, :, :, 1])
            nc.vector.tensor_add(tbm, mm[:, :, :, 2], mm[:, :, :, 3])
            if not last:
                p = sm.tile([P, R, 2, 8, 16], f16, tag="p")
                nc.vector.tensor_add(
                    p[:].rearrange("p r pa t ik -> p (r pa) t ik"), tam, tbm)
            else:
                p = grp.tile([P, R, 8, 2, 16], f16, tag="p3")
                for pa in (0, 1):
                    nc.vector.tensor_add(p[:, :, :, pa], ta[:, :, pa],
                                         tb[:, :, pa])
            return p

        def routing(g):
            up1 = st[g]["up1"]
            p1 = up1[:, :, :, :, 4, :]                  # [P,R,2,8,16] view
            md1, s1, sq11 = squash_a(g, p1[:, :, 0], p1[:, :, 1], 1)
            ab1 = wp_ab(g, p1)                          # sqrt runs under wp/ab
            f1 = squash_b(g, md1, s1, sq11)
            badd(g, ab1, f1, first=True)
            p2 = sig_p(g, last=False)
            md2, s2, sq12 = squash_a(g, p2[:, :, 0], p2[:, :, 1], 2)
            ab2 = wp_ab(g, p2)
            f2 = squash_b(g, md2, s2, sq12)
            badd(g, ab2, f2, first=False)
            p3 = sig_p(g, last=True)                    # [P,R,8,2,16] t-major
            md3, s3, sq13 = squash_a(g, p3[:, :, :, 0], p3[:, :, :, 1], 3)
            f3 = squash_b(g, md3, s3, sq13)
            v3 = grp.tile([P, R, 8, 2, 16], f16, tag="v3")
            for pa in (0, 1):
                nc.vector.tensor_mul(
                    v3[:, :, :, pa], p3[:, :, :, pa],
                    f3[:, :, pa].unsqueeze(3).broadcast_to((P, R, 8, 16)))
            nc.sync.dma_start(
                out=out_d[:, g * R * 256:(g + 1) * R * 256],
                in_=v3[:].rearrange("p a b c d -> p (a b c d)"))
            del st[g]

        for g in range(NG):
            mm_evac(g)
            prefix(g)
            routing(g)

    nc.compile()
    return nc


def _make_in_map(core, shards, w_in, wb_in):
    rb = (core % 4) * 32
    cxy_in = np.zeros((128, 1 + ROWS), np.float32)
    cxy_in[:, 0] = np.arange(128, dtype=np.float32) / 128.0
    cxy_in[:, 1:] = ((rb + np.arange(ROWS, dtype=np.float32)) / 128.0)[None, :]
    return {
        "x_shard": shards[core].astype(np.float16),
        "w_eff": w_in.astype(np.float16),
        "bapp": wb_in.astype(np.float16),
        "cxy": cxy_in,
    }


def kernel(x, W_conv, W_pos, W_app, b_app):
    from concourse.bass_utils import run_bass_kernel_spmd

    if "nc" not in _CACHE:
        _CACHE["nc"] = _build_module()
    nc = _CACHE["nc"]

    w_in, wb_in = _build_weights(W_conv, W_pos, W_app, b_app)
    shards = _shard_x(x)
    in_maps = [_make_in_map(core, shards, w_in, wb_in) for core in range(8)]

    trace = bool(int(os.environ.get("CAPS_TRACE", "0")))
    res = run_bass_kernel_spmd(nc, in_maps, core_ids=list(range(8)), trace=trace)
    _CACHE["last_result"] = res

    out = np.zeros((N, T1, Z, H, W), np.float32)
    for core in range(8):
        n, rb = core // 4, (core % 4) * 32
        o = res.results[core]["out_shard"].astype(np.float32).reshape(
            128, ROWS, 8, 2, 16)
        # [w, r, t, pa, ik] -> [t, pa*16+ik, r, w]
        out[n, :, :, rb:rb + 32, :] = o.transpose(2, 3, 4, 1, 0).reshape(
            8, 32, ROWS, 128)
    return out


# revision 28
# speedup vs baseline: 1.0719x; 1.0719x over previous
"""Trainium2 Bass kernel for nn_Caps2dMatwo (capsule conv + dual routing).

Strategy (8 NeuronCores, no cross-core communication):
  - Shard: core k handles batch n=k//4, H-rows [32*(k%4), 32*(k%4)+32).
  - PE stage: 3x3 conv + capsule pose/appearance transforms fused into 9
    per-tap matmuls over a permuted 128-channel axis, block-diagonal per
    32-row i-block.  PSUM column layout per i-block: (pa, t, c', k) with
    c'<4 the per-input-capsule u_hat and c'=4 the 0.5*sum_c partial (the
    iteration-1 routing p), then 40 cols of raw j=3 conv taps (t, c'')
    for the coordinate-addition fixup.
  - Routing (3 iterations) runs with R=8 rows batched per instruction:
    pixels on partitions, (row, pa, t, c, ik) on the free axis.  DVE does
    the heavy elementwise work, GPSIMD evacuates PSUM (folding the app
    bias into the evac add), ACT does only Sqrt/Sigmoid.
  - Agreements use raw p with factor scaling deferred past the reduce,
    so the ACT sqrt hides behind the big DVE ops.
  - Output is written w-major per core; the host transposes to
    channel-major when gathering.
"""
import os
from contextlib import ExitStack

import numpy as np

# problem constants (hardcoded per spec)
N, T0, T1 = 2, 4, 8
H, W = 128, 128
PD, AD = 4, 4
Z = 32
NBLK = 360          # psum cols per i-block: 320 (pa,t,c',k) + 40 (t,c'')
ROWS = 32           # output rows per core
P = 128
R = 8               # rows batched per routing group
NG = ROWS // R

_CACHE = {}


# ----------------------------------------------------------------- host prep
def _build_weights(W_conv, W_pos, W_app, b_app):
    """W_eff for the fused conv+pose matmul.

    Column layout within each i-block (360 cols):
      pa*160 + t*20 + c*4 + k   (c<4)  u_hat contribution
      pa*160 + t*20 + 16  + k          0.5 * sum_c u_hat  (iter-1 p)
      320 + t*5 + c             (c<4)  raw j=3 pos conv tap (coord fixup)
      320 + t*5 + 4                    0.5 * sum_c raw tap (p fixup)

    Returns:
      w_in [128, 9, 360]  per-partition weights, partition = i*32+c*8+q*4+j
      bapp [8, 5, 4]      app bias (t, c', k): c'<4 = b_app*colsum(Mapp),
                          c'=4 = 0.5*sum_c of that
    """
    Kc = np.asarray(W_conv, np.float64)[:, :, :, 0, :]          # [c,dy,dx,t1]
    Mpos = np.asarray(W_pos, np.float64).reshape(T0, T1, PD, PD).copy()
    Mpos = Mpos / np.sqrt(np.maximum((Mpos ** 2).sum(axis=2, keepdims=True), 1e-12))
    Mapp = np.asarray(W_app, np.float64).reshape(T0, T1, AD, AD)
    Sapp = Mapp.sum(axis=2)                                      # [c,t,k]

    W_eff = np.zeros((9, 128, 4, NBLK), np.float64)
    for tap in range(9):
        dy, dx = tap // 3, tap % 3
        for i in range(4):
            for c in range(4):
                for q in range(2):
                    for j in range(4):
                        row = i * 32 + c * 8 + q * 4 + j
                        for t in range(q, 8, 2):
                            kpos = Kc[c, dy, dx, t // 2]
                            kapp = Kc[c, dy, dx, 4 + t // 2]
                            for k in range(4):
                                W_eff[tap, row, i, t * 20 + c * 4 + k] = \
                                    kpos * Mpos[c, t, j, k]
                                W_eff[tap, row, i, 160 + t * 20 + c * 4 + k] = \
                                    kapp * Mapp[c, t, j, k]
                                W_eff[tap, row, i, t * 20 + 16 + k] += \
                                    0.5 * kpos * Mpos[c, t, j, k]
                                W_eff[tap, row, i, 160 + t * 20 + 16 + k] += \
                                    0.5 * kapp * Mapp[c, t, j, k]
                            if j == 3:
                                W_eff[tap, row, i, 320 + t * 5 + c] = kpos
                                W_eff[tap, row, i, 320 + t * 5 + 4] += 0.5 * kpos
    # [9, 128, 4, 360] -> [128, 9, 360] picking each partition's own block
    w_in = np.zeros((128, 9, NBLK), np.float32)
    for i in range(4):
        w_in[i * 32:(i + 1) * 32] = W_eff[:, i * 32:(i + 1) * 32, i, :].transpose(1, 0, 2)
    # bias "ones tap" weights: app-half columns get bias/32 (32 partitions
    # of ones contract to exactly bias; /32 is exact in fp16)
    buh = np.einsum('ct,ctk->tck', np.asarray(b_app, np.float64), Sapp)  # [t,c,k]
    wb = np.zeros((NBLK,), np.float64)
    for t in range(8):
        for c in range(4):
            wb[160 + t * 20 + c * 4:160 + t * 20 + c * 4 + 4] = buh[t, c]
        wb[160 + t * 20 + 16:160 + t * 20 + 20] = 0.5 * buh[t].sum(axis=0)
    wb_in = np.broadcast_to((wb / 32.0)[None, :], (128, NBLK))
    return w_in, np.ascontiguousarray(wb_in).astype(np.float32)


def _shard_x(x):
    """x [N,T0,Z,H,W] -> list of 8 arrays [128, 34*130] (permuted channels)."""
    xp = np.zeros((N, T0, Z, H + 2, W + 2), np.float32)
    xp[:, :, :, 1:H + 1, 1:W + 1] = np.asarray(x, np.float32)
    # z = q*16 + i*4 + j ; partition = i*32 + c*8 + q*4 + j
    xq = xp.reshape(N, T0, 2, 4, 4, H + 2, W + 2)                # n c q i j h w
    xperm = np.ascontiguousarray(xq.transpose(0, 3, 1, 2, 4, 5, 6)
                                 ).reshape(N, 128, H + 2, W + 2)
    shards = []
    for core in range(8):
        n, rb = core // 4, (core % 4) * 32
        shards.append(np.ascontiguousarray(
            xperm[n, :, rb:rb + 34, :]).reshape(128, 34 * 130))
    return shards


# ------------------------------------------------------------- bass module
def _build_module():
    import concourse.bass as bass
    import concourse.tile as tile
    import concourse.mybir as mybir
    from concourse import bacc

    f32 = mybir.dt.float32
    f16 = mybir.dt.float16
    AX = mybir.AxisListType.X
    OP = mybir.AluOpType
    AF = mybir.ActivationFunctionType

    nc = bacc.Bacc("TRN2", num_devices=8, debug=False)
    x_d = nc.dram_tensor("x_shard", [128, 34 * 130], f16, kind="ExternalInput").ap()
    w_d = nc.dram_tensor("w_eff", [128, 9, NBLK], f16, kind="ExternalInput").ap()
    bapp_d = nc.dram_tensor("bapp", [128, NBLK], f16, kind="ExternalInput").ap()
    cxy_d = nc.dram_tensor("cxy", [128, 1 + ROWS], f32, kind="ExternalInput").ap()
    out_d = nc.dram_tensor("out_shard", [128, ROWS * 256], f16,
                           kind="ExternalOutput").ap()

    with tile.TileContext(nc) as tc, ExitStack() as ctx:
        const = ctx.enter_context(tc.tile_pool(name="const", bufs=1))
        grp = ctx.enter_context(tc.tile_pool(name="grp", bufs=2))
        sm = ctx.enter_context(tc.tile_pool(name="sm", bufs=2))
        big = ctx.enter_context(tc.tile_pool(name="big", bufs=1))
        psum = ctx.enter_context(tc.tile_pool(name="psum", bufs=2, space="PSUM"))

        x_sb = const.tile([P, 34, 130], f16)
        nc.sync.dma_start(out=x_sb[:].rearrange("p a b -> p (a b)"), in_=x_d)
        w_sb = const.tile([P, 9, NBLK], f16)
        nc.sync.dma_start(out=w_sb, in_=w_d)
        wb_sb = const.tile([P, NBLK], f16)        # bias/32 "ones tap" weights
        nc.sync.dma_start(out=wb_sb, in_=bapp_d)
        cxy = const.tile([P, 1 + ROWS], f32)      # [cx | cy per row]
        nc.sync.dma_start(out=cxy, in_=cxy_d)
        eps_t = const.tile([P, 1], f32)
        nc.vector.memset(eps_t, 1e-9)
        ones_sb = const.tile([P, 128], f16)
        nc.vector.memset(ones_sb, 1.0)

        st = {}  # per-group live tiles

        def mm_evac(g):
            """PE matmuls + ACT PSUM evacuation for group g's rows."""
            up1 = grp.tile([P, R, 2, 8, 5, 16], f16, tag="up1")
            crw = grp.tile([P, R, 8, 5, 4], f16, tag="crw")
            st[g] = {"up1": up1, "crw": crw}
            for j in range(R):
                r = g * R + j
                ps = psum.tile([P, 4, 512], f32, tag="ps")
                for tap in range(9):
                    dy, dx = tap // 3, tap % 3
                    patch = x_sb[:, r + dy, dx:dx + 128]
                    for i in range(4):
                        nc.tensor.matmul(
                            ps[:, i, 0:NBLK],
                            lhsT=patch[32 * i:32 * (i + 1), :],
                            rhs=w_sb[32 * i:32 * (i + 1), tap, :],
                            start=(tap == 0), stop=False,
                            tile_position=(32 * i, 0))
                for i in range(4):      # bias tap: sum_p 1*(bias/32) = bias
                    nc.tensor.matmul(
                        ps[:, i, 0:NBLK],
                        lhsT=ones_sb[32 * i:32 * (i + 1), :],
                        rhs=wb_sb[32 * i:32 * (i + 1), :],
                        start=False, stop=True,
                        tile_position=(32 * i, 0))
                # evac: (i, (t c'), k) views — 3 free dims on both sides
                src = ps[:, :, 0:320].rearrange("p i (pa tc k) -> p pa i tc k",
                                                pa=2, tc=40)
                dst = up1[:, j].rearrange("p pa t c (i k) -> p pa i (t c) k", i=4)
                nc.scalar.copy(dst[:, 0], src[:, 0])
                nc.scalar.copy(dst[:, 1], src[:, 1])
                nc.scalar.copy(
                    crw[:, j],
                    ps[:, :, 320:360].rearrange("p i (t c) -> p t c i", t=8))

        def prefix(g):
            """GPSIMD: coordinate-addition fixup into u_hat pos (k=0,1).

            The c'=4 (p1) slot rides along: tmp's c''=4 holds the scaled
            0.5*sum_c raw tap, exactly the p1 coordinate term.  All TT ops
            (TensorScalarPtr is ~100x slower on GPSIMD).
            """
            up1, crw = st[g]["up1"], st[g]["crw"]
            tmp = sm.tile([P, 2, R, 8, 5, 4], f16, tag="tmp")
            crf = crw[:].rearrange("p r t c i -> p (r t c i)")
            nc.gpsimd.tensor_mul(
                tmp[:, 0].rearrange("p r t c i -> p (r t c i)"),
                crf, cxy[:, 0:1].broadcast_to((P, R * 160)))
            cyg = (cxy[:, 1 + g * R:1 + (g + 1) * R]
                   .rearrange("p r -> p r ()")
                   .broadcast_to((P, R, 160)))
            nc.gpsimd.tensor_mul(
                tmp[:, 1].rearrange("p r t c i -> p r (t c i)"),
                crw[:].rearrange("p r t c i -> p r (t c i)"), cyg)
            uh_ik = up1[:, :, 0].rearrange("p r t c (i k) -> p r (t c) i k", i=4)
            for k in (0, 1):
                nc.gpsimd.tensor_add(
                    uh_ik[:, :, :, :, k], uh_ik[:, :, :, :, k],
                    tmp[:, k].rearrange("p r t c i -> p r (t c) i"))

        def squash_mx_sq(g, ppos, papp, it):
            """mx reduce (DVE) + square (GPSIMD for iters 1-2)."""
            md = sm.tile([P, R, 2, 8], f32, tag=f"md{it}")
            nc.vector.tensor_reduce(out=md[:, :, 0], in_=ppos, axis=AX,
                                    op=OP.max, apply_absolute_value=True)
            sq = big.tile([P, R, 8, 16], f32, tag="sq")
            eng = nc.gpsimd if it < 3 else nc.vector
            eng.tensor_mul(sq, papp, papp)
            return md, sq

        def squash_s(g, sq, it):
            """s reduce (DVE) + ACT sqrt issue."""
            s = sm.tile([P, R, 8], f32, tag=f"s{it}")
            nc.vector.tensor_reduce(out=s, in_=sq, axis=AX, op=OP.add)
            sq1 = sm.tile([P, R, 8], f32, tag=f"sq1{it}")
            nc.scalar.activation(sq1, s, AF.Sqrt, bias=eps_t[:, 0:1])
            return s, sq1

        def squash_b(g, md, s, sq1):
            """DVE: den = (1+s)*sqrt, f = [1/mx | s/den]."""
            nc.vector.scalar_tensor_tensor(out=md[:, :, 1], in0=s, scalar=1.0,
                                           in1=sq1, op0=OP.add, op1=OP.mult)
            f = sm.tile([P, R, 2, 8], f16, tag="f")
            with nc.allow_low_precision("f consumed in fp16 muls"):
                nc.vector.reciprocal(f[:], md[:])
            nc.vector.tensor_mul(f[:, :, 1], s, f[:, :, 1])
            return f

        def wp_wa(g, pv):
            """DVE: wp = uh*p (bcast over c), first tree level."""
            up1 = st[g]["up1"]
            uh = (up1[:, :, :, :, 0:4, :]
                  .rearrange("p r pa t c ik -> p (r pa t) c ik"))
            pm = (pv.rearrange("p r pa t ik -> p (r pa t) () ik")
                  .broadcast_to((P, R * 16, 4, 16)))
            wp = big.tile([P, R * 16, 4, 16], f16, tag="wp")
            nc.vector.tensor_mul(wp, uh, pm)
            wa = big.tile([P, R * 16, 4, 8], f16, tag="wa")
            nc.vector.tensor_add(wa, wp[:, :, :, 0:8], wp[:, :, :, 8:16])
            return wa

        def wbc_ab(g, wa):
            """DVE: remaining tree levels -> ab = sum_ik uh*p."""
            wb = big.tile([P, R * 16, 4, 4], f16, tag="wb")
            nc.vector.tensor_add(wb, wa[:, :, :, 0:4], wa[:, :, :, 4:8])
            wc = big.tile([P, R * 16, 4, 2], f16, tag="wc")
            nc.vector.tensor_add(wc, wb[:, :, :, 0:2], wb[:, :, :, 2:4])
            ab = sm.tile([P, R * 16, 4], f32, tag="ab")
            nc.vector.tensor_add(ab, wc[:, :, :, 0], wc[:, :, :, 1])
            return ab

        def badd(g, ab, f, first):
            """DVE: rt = (f_pos*ab_pos)*(f_app*ab_app); b (+)= rt."""
            t1 = sm.tile([P, R, 2, 8, 4], f16, tag="t1")
            nc.vector.tensor_mul(
                t1[:].rearrange("p r pa t c -> p (r pa) t c"),
                ab[:].rearrange("p (rpa t) c -> p rpa t c", t=8),
                f[:].rearrange("p r pa t -> p (r pa) t ()")
                .broadcast_to((P, R * 2, 8, 4)))
            if first:
                b = sm.tile([P, R, 8, 4], f32, tag="b")
                st[g]["b"] = b
                nc.vector.tensor_mul(b, t1[:, :, 0], t1[:, :, 1])
            else:
                b = st[g]["b"]
                rt = sm.tile([P, R, 8, 4], f32, tag="rt")
                nc.vector.tensor_mul(rt, t1[:, :, 0], t1[:, :, 1])
                nc.vector.tensor_add(b, b, rt)

        def sig_p(g, last):
            """ACT sigmoid (ik-expanded) + DVE: p = sum_c uh*r.

            Returns p as [P,R,2,8,16] (pa-major), or for `last` as
            [P,R,8,2,16] (t-major, the output channel layout).
            """
            up1, b = st[g]["up1"], st[g]["b"]
            # sigmoid with 4x inner expansion (ACT), then 4x tile-out on
            # DVE with the broadcast on a middle dim (stride-0 inner kills
            # DVE/ACT throughput; extent-4 inner bcast is tolerable)
            r2a = sm.tile([P, R, 8, 4, 4], f16, tag="r2a")
            nc.scalar.activation(
                r2a[:].rearrange("p r t c k -> p (r t c) k"),
                b[:].rearrange("p r t c -> p (r t c) ()")
                .broadcast_to((P, R * 32, 4)), AF.Sigmoid)
            r2x = big.tile([P, R * 32, 4, 4], f16, tag="r2x")
            nc.vector.tensor_scalar_mul(
                r2x, r2a[:].rearrange("p r t c k -> p (r t c) () k")
                .broadcast_to((P, R * 32, 4, 4)), 1.0)
            m = big.tile([P, R, 2, 8, 4, 16], f16, tag="m")
            for pa in (0, 1):
                nc.vector.tensor_mul(
                    m[:, :, pa].rearrange("p r t c ik -> p r t (c ik)"),
                    up1[:, :, pa, :, 0:4, :].rearrange(
                        "p r t c ik -> p r t (c ik)"),
                    r2x[:].rearrange("p (r t c) rep k -> p r t (c rep k)",
                                     r=R, t=8))
            mm = m[:].rearrange("p r pa t c ik -> p (r pa) t c ik")
            ta = big.tile([P, R, 2, 8, 16], f16, tag="ta")
            tb = big.tile([P, R, 2, 8, 16], f16, tag="tb")
            tam = ta[:].rearrange("p r pa t ik -> p (r pa) t ik")
            tbm = tb[:].rearrange("p r pa t ik -> p (r pa) t ik")
            nc.vector.tensor_add(tam, mm[:, :, :, 0], mm[:, :, :, 1])
            nc.vector.tensor_add(tbm, mm[:, :, :, 2], mm[:, :, :, 3])
            if not last:
                p = sm.tile([P, R, 2, 8, 16], f16, tag="p")
                nc.vector.tensor_add(
                    p[:].rearrange("p r pa t ik -> p (r pa) t ik"), tam, tbm)
            else:
                p = grp.tile([P, R, 8, 2, 16], f16, tag="p3")
                for pa in (0, 1):
                    nc.vector.tensor_add(p[:, :, :, pa], ta[:, :, pa],
                                         tb[:, :, pa])
            return p

        def routing(g):
            up1 = st[g]["up1"]
            p1 = up1[:, :, :, :, 4, :]                  # [P,R,2,8,16] view
            # interleave: wp/wa cover the GPSIMD square, wb/wc/ab cover
            # the ACT sqrt (+table load), keeping the in-order DVE queue
            # from stalling on either.
            md1, g1 = squash_mx_sq(g, p1[:, :, 0], p1[:, :, 1], 1)
            wa1 = wp_wa(g, p1)
            s1, sq11 = squash_s(g, g1, 1)
            ab1 = wbc_ab(g, wa1)
            f1 = squash_b(g, md1, s1, sq11)
            badd(g, ab1, f1, first=True)
            p2 = sig_p(g, last=False)
            md2, g2 = squash_mx_sq(g, p2[:, :, 0], p2[:, :, 1], 2)
            wa2 = wp_wa(g, p2)
            s2, sq12 = squash_s(g, g2, 2)
            ab2 = wbc_ab(g, wa2)
            f2 = squash_b(g, md2, s2, sq12)
            badd(g, ab2, f2, first=False)
            p3 = sig_p(g, last=True)                    # [P,R,8,2,16] t-major
            md3, g3 = squash_mx_sq(g, p3[:, :, :, 0], p3[:, :, :, 1], 3)
            s3, sq13 = squash_s(g, g3, 3)
            f3 = squash_b(g, md3, s3, sq13)
            v3 = grp.tile([P, R, 8, 2, 16], f16, tag="v3")
            for pa in (0, 1):
                nc.vector.tensor_mul(
                    v3[:, :, :, pa], p3[:, :, :, pa],
                    f3[:, :, pa].unsqueeze(3).broadcast_to((P, R, 8, 16)))
            nc.sync.dma_start(
                out=out_d[:, g * R * 256:(g + 1) * R * 256],
                in_=v3[:].rearrange("p a b c d -> p (a b c d)"))
            del st[g]

        for g in range(NG):
            mm_evac(g)
            prefix(g)
            routing(g)

    nc.compile()
    return nc


def _make_in_map(core, shards, w_in, wb_in):
    rb = (core % 4) * 32
    cxy_in = np.zeros((128, 1 + ROWS), np.float32)
    cxy_in[:, 0] = np.arange(128, dtype=np.float32) / 128.0
    cxy_in[:, 1:] = ((rb + np.arange(ROWS, dtype=np.float32)) / 128.0)[None, :]
    return {
        "x_shard": shards[core].astype(np.float16),
        "w_eff": w_in.astype(np.float16),
        "bapp": wb_in.astype(np.float16),
        "cxy": cxy_in,
    }


def kernel(x, W_conv, W_pos, W_app, b_app):
    from concourse.bass_utils import run_bass_kernel_spmd

    if "nc" not in _CACHE:
        _CACHE["nc"] = _build_module()
    nc = _CACHE["nc"]

    w_in, wb_in = _build_weights(W_conv, W_pos, W_app, b_app)
    shards = _shard_x(x)
    in_maps = [_make_in_map(core, shards, w_in, wb_in) for core in range(8)]

    trace = bool(int(os.environ.get("CAPS_TRACE", "0")))
    res = run_bass_kernel_spmd(nc, in_maps, core_ids=list(range(8)), trace=trace)
    _CACHE["last_result"] = res

    out = np.zeros((N, T1, Z, H, W), np.float32)
    for core in range(8):
        n, rb = core // 4, (core % 4) * 32
        o = res.results[core]["out_shard"].astype(np.float32).reshape(
            128, ROWS, 8, 2, 16)
        # [w, r, t, pa, ik] -> [t, pa*16+ik, r, w]
        out[n, :, :, rb:rb + 32, :] = o.transpose(2, 3, 4, 1, 0).reshape(
            8, 32, ROWS, 128)
    return out
